# revision 2
# baseline (speedup 1.0000x reference)
"""Chorus (nn_Chorus_73160472920641) Trainium2 Bass kernel.

out[b,t] = 0.5*x[b,t] + 0.25*(x[b,t-d0(t)] + x[b,t-d1(t)])   (0 for t-d<0)

Structure exploited:
- d_v(t) is a static table, nearly periodic with period P=29400 samples;
  d1 == d0 rotated by P/2 (up to a handful of +-1 trunc mismatches that we
  patch with a few masked 1-column DVE ops).
- Layout: units = half-periods (14700 samples). Partition = (row, unit).
  Every unit needs gathers with BOTH half-tables, so all 128 partitions of
  a tile share the same static gather structure.
- The gather decomposes into ~441 constant-delay runs per half-table; each
  run is a shifted contiguous copy -> tiny scaled-identity matmul on the
  TensorEngine accumulating 0.25*g0 + 0.25*g1 in PSUM (one stationary
  0.25*I, loaded once). DVE drains PSUM fused with the 0.5*x dry path via
  the AFFINE_THEN_ADD custom op.
- Pure data parallel over batch: 16 rows -> 8 cores x 2 rows.
"""

import sys

import numpy as np

sys.path.insert(0, "/opt/trn_rl_repo")

import concourse.bacc as bacc
import concourse.mybir as mybir
import concourse.tile as tile
from concourse.ap import AP
from concourse.bass_utils import run_bass_kernel_spmd

SR = 44100
RATE = 1.5
B, T_FULL = 16, 2646000
P = 29400
HALF = 14700
HALO = 1102
CHUNK = 4900
BLK = 490
N_CORES = 8
PARTS = 128


def _delay_table(T):
    base = int(20.0 * SR / 1000)
    rng = int(10.0 * SR / 1000 * 0.5)
    t = np.arange(T, dtype=np.float64)[None, :]
    ph0 = (np.arange(2, dtype=np.float64) / 2)[:, None]
    phase = (ph0 + t * RATE / SR) % 1.0
    mod = np.sin(2.0 * np.pi * phase)
    delay = base + (mod * rng).astype(np.int64)
    return np.clip(delay, 1, 2047)


def _plan(nper):
    """Static plan: run lists per section/block, patch groups, tiles."""
    T = nper * P
    units = 2 * nper
    delay = _delay_table(T)
    tbl = delay[0, :P].copy()

    # runs per section, split at BLK boundaries
    runs = [[], []]  # section -> list of (o, ln, src_col)
    for s in (0, 1):
        ts = tbl[s * HALF : (s + 1) * HALF]
        bnd = [0] + list(np.nonzero(np.diff(ts))[0] + 1) + [HALF]
        for a, b in zip(bnd[:-1], bnd[1:]):
            d = int(ts[a])
            # split at BLK boundaries
            o = a
            while o < b:
                e = min(b, (o // BLK + 1) * BLK)
                runs[s].append((o, e - o, o + HALO - d))
                o = e
    runs_by_block = [[[] for _ in range(HALF // BLK)] for _ in (0, 1)]
    for s in (0, 1):
        for o, ln, src in runs[s]:
            runs_by_block[s][o // BLK].append((o, ln, src))

    # patch groups: (o, sec_used, diff) -> set of units
    u_of_t = np.arange(T) // HALF
    o_of_t = np.arange(T) % HALF
    groups = {}
    for role in (0, 1):
        sec = (u_of_t + role) % 2
        used = tbl[sec * HALF + o_of_t]
        dv = delay[role]
        bad = np.nonzero(used != dv)[0]
        for t in bad:
            key = (int(o_of_t[t]), int(sec[t]), int(dv[t] - used[t]))
            groups.setdefault(key, {})
            u = int(u_of_t[t])
            groups[key][u] = groups[key].get(u, 0.0) + 0.25
    for (o, s, diff), _ in groups.items():
        col = o + HALO - int(tbl[s * HALF + o])
        assert 0 <= col - diff < HALO + HALF, (o, s, diff, col)

    # tiles: (h0, h_store0, nh)
    nh = min(64, units)
    tiles = []
    h0, stored = 0, 0
    while stored < units:
        h0 = min(h0, units - nh)
        tiles.append((h0, stored, nh))
        stored = h0 + nh
        h0 = stored
    return T, units, tiles, runs_by_block, groups, nh


def _masks_for_tiles(tiles, groups, nh):
    """Per tile, ordered patch list [(o, sec, diff, col, gidx)] and the
    concatenated mask tensor [128, n_groups_total]."""
    tile_patches = []
    cols = []
    tbl = None
    for h0, _, _ in tiles:
        plist = []
        for (o, s, diff), umask in sorted(groups.items()):
            m = np.zeros((PARTS, 1), np.float32)
            hit = False
            for r in (0, 1):
                for i in range(nh):
                    u = h0 + i
                    if u in umask:
                        m[r * 64 + i, 0] = umask[u]
                        hit = True
            if hit:
                plist.append((o, s, diff, len(cols)))
                cols.append(m)
        tile_patches.append(plist)
    msk = np.concatenate(cols, axis=1) if cols else np.zeros((PARTS, 1), np.float32)
    return tile_patches, msk


def build(nper):
    T, units, tiles, runs_by_block, groups, nh = _plan(nper)
    delay = _delay_table(T)
    tbl = delay[0, :P]
    tile_patches, msk_np = _masks_for_tiles(tiles, groups, nh)

    nc = bacc.Bacc("TRN2", target_bir_lowering=False, debug=False)
    x = nc.dram_tensor("x", [2, T], mybir.dt.float32, kind="ExternalInput")
    w = nc.dram_tensor("w", [PARTS, PARTS], mybir.dt.float32, kind="ExternalInput")
    mk = nc.dram_tensor("msk", list(msk_np.shape), mybir.dt.float32, kind="ExternalInput")
    y = nc.dram_tensor("y", [2, T], mybir.dt.float32, kind="ExternalOutput")

    wlen = HALO + HALF
    nblk = HALF // BLK
    nchunk = HALF // CHUNK
    bpc = CHUNK // BLK

    with tile.TileContext(nc) as tc:
        with (
            tc.tile_pool(name="wp", bufs=1) as wp,
            tc.tile_pool(name="inp", bufs=2) as inp,
            tc.tile_pool(name="outp", bufs=2) as outp,
            tc.tile_pool(name="ps", bufs=8, space="PSUM") as ps,
            tc.tile_pool(name="tp", bufs=4) as tp,
        ):
            wt = wp.tile([PARTS, PARTS], mybir.dt.float32, tag="wt")
            nc.sync.dma_start(wt[:], w.ap())
            mkt = wp.tile(list(msk_np.shape), mybir.dt.float32, tag="mk")
            nc.sync.dma_start(mkt[:], mk.ap())

            for ti, (h0, hs0, nh_t) in enumerate(tiles):
                in_t = inp.tile([PARTS, wlen], mybir.dt.float32, tag="in")
                if nh_t < 64:
                    nc.gpsimd.memset(in_t[:], 0.0)
                for r in (0, 1):
                    p0 = r * 64
                    if h0 == 0:
                        nc.gpsimd.memset(in_t[p0 : p0 + 1, 0:HALO], 0.0)
                        nc.sync.dma_start(
                            in_t[p0 : p0 + 1, HALO:wlen],
                            AP(x, r * T, [[HALF, 1], [1, HALF]]),
                        )
                        if nh_t > 1:
                            nc.sync.dma_start(
                                in_t[p0 + 1 : p0 + nh_t, :],
                                AP(x, r * T + HALF - HALO, [[HALF, nh_t - 1], [1, wlen]]),
                            )
                    else:
                        nc.sync.dma_start(
                            in_t[p0 : p0 + nh_t, :],
                            AP(x, r * T + h0 * HALF - HALO, [[HALF, nh_t], [1, wlen]]),
                        )
                for c in range(nchunk):
                    out_t = outp.tile([PARTS, CHUNK], mybir.dt.float32, tag="out")
                    for bb in range(bpc):
                        blk_lo = c * CHUNK + bb * BLK
                        pt = ps.tile([PARTS, BLK], mybir.dt.float32, tag="ps")
                        mms = []
                        for s in (0, 1):
                            mms += runs_by_block[s][c * bpc + bb]
                        for k, (o, ln, src) in enumerate(mms):
                            nc.tensor.matmul(
                                pt[:, o - blk_lo : o - blk_lo + ln],
                                wt[:],
                                in_t[:, src : src + ln],
                                start=(k == 0),
                                stop=(k == len(mms) - 1),
                                skip_group_check=True,
                            )
                        nc.vector.affine_then_add(
                            out=out_t[:, bb * BLK : (bb + 1) * BLK],
                            in0=in_t[:, HALO + blk_lo : HALO + blk_lo + BLK],
                            in1=pt[:],
                            scale=0.5,
                            bias=0.0,
                        )
                    # patches for this chunk
                    for o, s, diff, gidx in tile_patches[ti]:
                        if not (c * CHUNK <= o < (c + 1) * CHUNK):
                            continue
                        col = o + HALO - int(tbl[s * HALF + o])
                        t1 = tp.tile([PARTS, 1], mybir.dt.float32, tag="t1")
                        t2 = tp.tile([PARTS, 1], mybir.dt.float32, tag="t2")
                        nc.vector.tensor_tensor(
                            out=t1[:],
                            in0=in_t[:, col - diff : col - diff + 1],
                            in1=in_t[:, col : col + 1],
                            op=mybir.AluOpType.subtract,
                        )
                        nc.vector.tensor_tensor(
                            out=t2[:], in0=t1[:], in1=mkt[:, gidx : gidx + 1],
                            op=mybir.AluOpType.mult,
                        )
                        oc = o - c * CHUNK
                        nc.vector.tensor_tensor(
                            out=out_t[:, oc : oc + 1],
                            in0=out_t[:, oc : oc + 1],
                            in1=t2[:],
                            op=mybir.AluOpType.add,
                        )
                    # store
                    for r in (0, 1):
                        skip = hs0 - h0
                        n_st = nh_t - skip
                        nc.sync.dma_start(
                            AP(y, r * T + hs0 * HALF + c * CHUNK, [[HALF, n_st], [1, CHUNK]]),
                            out_t[r * 64 + skip : r * 64 + nh_t, :],
                        )
    nc.compile()
    return nc, msk_np


_CACHE = {}


def _get_built(nper):
    if nper not in _CACHE:
        _CACHE[nper] = build(nper)
    return _CACHE[nper]


def kernel(x):
    x = np.asarray(x, dtype=np.float32)
    assert x.shape == (B, T_FULL)
    nper = T_FULL // P
    nc, msk_np = _get_built(nper)
    wv = (0.25 * np.eye(PARTS)).astype(np.float32)
    in_maps = [
        {"x": np.ascontiguousarray(x[2 * i : 2 * i + 2]), "w": wv, "msk": msk_np}
        for i in range(N_CORES)
    ]
    res = run_bass_kernel_spmd(nc, in_maps, core_ids=list(range(N_CORES)))
    out = np.concatenate([r["y"] for r in res.results], axis=0)
    return out.astype(np.float32)


if __name__ == "__main__":
    # smoke test on a small number of periods through CoreSim
    from concourse.bass_interp import CoreSim

    nper = 2
    T = nper * P
    nc, msk_np = build(nper)
    rng = np.random.default_rng(0)
    xv = rng.standard_normal((2, T)).astype(np.float32)
    sim = CoreSim(nc, trace=False)
    sim.tensor("x")[:] = xv
    sim.tensor("w")[:] = (0.25 * np.eye(PARTS)).astype(np.float32)
    sim.tensor("msk")[:] = msk_np
    sim.simulate()
    got = sim.tensor("y").copy()
    # reference
    delay = _delay_table(T)
    idx = np.arange(T)[None, :] - delay
    valid = (idx >= 0).astype(np.float32)
    idx = np.maximum(idx, 0)
    wet = (xv[:, idx] * valid[None]).mean(axis=1)
    exp = xv * 0.5 + wet * 0.5
    err = np.abs(got - exp).max()
    print("smoke absmax err:", err, "rel:", err / np.abs(exp).max())


# revision 4
# speedup vs baseline: 42514.6041x; 42514.6041x over previous
"""Chorus (nn_Chorus_73160472920641) Trainium2 Bass kernel.

out[b,t] = 0.5*x[b,t] + 0.25*(x[b,t-d0(t)] + x[b,t-d1(t)])   (0 for t-d<0)

Structure exploited:
- d_v(t) is a static table, nearly periodic with period P=29400 samples;
  d1 == d0 rotated by P/2 (up to a handful of +-1 trunc mismatches that we
  patch with a few masked 1-column DVE ops).
- Layout: units = half-periods (14700 samples). Partition = (row, unit).
  Every unit needs gathers with BOTH half-tables, so all 128 partitions of
  a tile share the same static gather structure.
- The gather decomposes into ~441 constant-delay runs per half-table; each
  run is a shifted contiguous copy -> tiny scaled-identity matmul on the
  TensorEngine accumulating 0.25*g0 + 0.25*g1 in PSUM (one stationary
  0.25*I, loaded once). DVE drains PSUM fused with the 0.5*x dry path via
  the AFFINE_THEN_ADD custom op.
- Pure data parallel over batch: 16 rows -> 8 cores x 2 rows.
"""

import sys

import numpy as np

sys.path.insert(0, "/opt/trn_rl_repo")

import concourse.bacc as bacc
import concourse.mybir as mybir
import concourse.tile as tile
from concourse.ap import AP
from concourse.bass_utils import run_bass_kernel_spmd

SR = 44100
RATE = 1.5
B, T_FULL = 16, 2646000
P = 29400
HALF = 14700
HALO = 1102
CHUNK = 4900
BLK = 490
N_CORES = 8
PARTS = 128


def _delay_table(T):
    base = int(20.0 * SR / 1000)
    rng = int(10.0 * SR / 1000 * 0.5)
    t = np.arange(T, dtype=np.float64)[None, :]
    ph0 = (np.arange(2, dtype=np.float64) / 2)[:, None]
    phase = (ph0 + t * RATE / SR) % 1.0
    mod = np.sin(2.0 * np.pi * phase)
    delay = base + (mod * rng).astype(np.int64)
    return np.clip(delay, 1, 2047)


def _plan(nper):
    """Static plan: run lists per section/block, patch groups, tiles."""
    T = nper * P
    units = 2 * nper
    delay = _delay_table(T)
    tbl = delay[0, :P].copy()

    # runs per section, split at BLK boundaries
    runs = [[], []]  # section -> list of (o, ln, src_col)
    for s in (0, 1):
        ts = tbl[s * HALF : (s + 1) * HALF]
        bnd = [0] + list(np.nonzero(np.diff(ts))[0] + 1) + [HALF]
        for a, b in zip(bnd[:-1], bnd[1:]):
            d = int(ts[a])
            # split at BLK boundaries
            o = a
            while o < b:
                e = min(b, (o // BLK + 1) * BLK)
                runs[s].append((o, e - o, o + HALO - d))
                o = e
    runs_by_block = [[[] for _ in range(HALF // BLK)] for _ in (0, 1)]
    for s in (0, 1):
        for o, ln, src in runs[s]:
            runs_by_block[s][o // BLK].append((o, ln, src))

    # patch groups: (o, sec_used, diff) -> set of units
    u_of_t = np.arange(T) // HALF
    o_of_t = np.arange(T) % HALF
    groups = {}
    for role in (0, 1):
        sec = (u_of_t + role) % 2
        used = tbl[sec * HALF + o_of_t]
        dv = delay[role]
        bad = np.nonzero(used != dv)[0]
        for t in bad:
            key = (int(o_of_t[t]), int(sec[t]), int(dv[t] - used[t]))
            groups.setdefault(key, {})
            u = int(u_of_t[t])
            groups[key][u] = groups[key].get(u, 0.0) + 0.25
    for (o, s, diff), _ in groups.items():
        col = o + HALO - int(tbl[s * HALF + o])
        assert 0 <= col - diff < HALO + HALF, (o, s, diff, col)

    # tiles: (h0, h_store0, nh)
    nh = min(64, units)
    tiles = []
    h0, stored = 0, 0
    while stored < units:
        h0 = min(h0, units - nh)
        tiles.append((h0, stored, nh))
        stored = h0 + nh
        h0 = stored
    return T, units, tiles, runs_by_block, groups, nh


def _masks_for_tiles(tiles, groups, nh):
    """Per tile, ordered patch list [(o, sec, diff, col, gidx)] and the
    concatenated mask tensor [128, n_groups_total]."""
    tile_patches = []
    cols = []
    tbl = None
    for h0, _, _ in tiles:
        plist = []
        for (o, s, diff), umask in sorted(groups.items()):
            m = np.zeros((PARTS, 1), np.float32)
            hit = False
            for r in (0, 1):
                for i in range(nh):
                    u = h0 + i
                    if u in umask:
                        m[r * 64 + i, 0] = umask[u]
                        hit = True
            if hit:
                plist.append((o, s, diff, len(cols)))
                cols.append(m)
        tile_patches.append(plist)
    msk = np.concatenate(cols, axis=1) if cols else np.zeros((PARTS, 1), np.float32)
    return tile_patches, msk


def build(nper):
    T, units, tiles, runs_by_block, groups, nh = _plan(nper)
    delay = _delay_table(T)
    tbl = delay[0, :P]
    tile_patches, msk_np = _masks_for_tiles(tiles, groups, nh)

    nc = bacc.Bacc("TRN2", target_bir_lowering=False, debug=False)
    x = nc.dram_tensor("x", [2, T], mybir.dt.float32, kind="ExternalInput")
    w = nc.dram_tensor("w", [PARTS, PARTS], mybir.dt.float32, kind="ExternalInput")
    mk = nc.dram_tensor("msk", list(msk_np.shape), mybir.dt.float32, kind="ExternalInput")
    y = nc.dram_tensor("y", [2, T], mybir.dt.float32, kind="ExternalOutput")

    wlen = HALO + HALF
    nblk = HALF // BLK
    nchunk = HALF // CHUNK
    bpc = CHUNK // BLK

    with tile.TileContext(nc) as tc:
        with (
            tc.tile_pool(name="wp", bufs=1) as wp,
            tc.tile_pool(name="inp", bufs=2) as inp,
            tc.tile_pool(name="outp", bufs=3) as outp,
            tc.tile_pool(name="ps", bufs=8, space="PSUM") as ps,
            tc.tile_pool(name="tp", bufs=4) as tp,
        ):
            wt = wp.tile([PARTS, PARTS], mybir.dt.float32, tag="wt")
            nc.sync.dma_start(wt[:], w.ap())
            mkt = wp.tile(list(msk_np.shape), mybir.dt.float32, tag="mk")
            nc.sync.dma_start(mkt[:], mk.ap())

            for ti, (h0, hs0, nh_t) in enumerate(tiles):
                in_t = inp.tile([PARTS, wlen], mybir.dt.float32, tag="in")
                if nh_t < 64:
                    nc.gpsimd.memset(in_t[:], 0.0)
                # chunk-aligned col windows so chunk 0 compute starts early
                wins = []
                lo = 0
                for c in range(nchunk):
                    hi = min(wlen, HALO + (c + 1) * CHUNK)
                    wins.append((lo, hi))
                    lo = hi
                for r in (0, 1):
                    p0 = r * 64
                    if h0 == 0:
                        nc.gpsimd.memset(in_t[p0 : p0 + 1, 0:HALO], 0.0)
                        nc.sync.dma_start(
                            in_t[p0 : p0 + 1, HALO:wlen],
                            AP(x, r * T, [[HALF, 1], [1, HALF]]),
                        )
                        if nh_t > 1:
                            for lo, hi in wins:
                                nc.sync.dma_start(
                                    in_t[p0 + 1 : p0 + nh_t, lo:hi],
                                    AP(x, r * T + HALF - HALO + lo, [[HALF, nh_t - 1], [1, hi - lo]]),
                                )
                    else:
                        for lo, hi in wins:
                            nc.sync.dma_start(
                                in_t[p0 : p0 + nh_t, lo:hi],
                                AP(x, r * T + h0 * HALF - HALO + lo, [[HALF, nh_t], [1, hi - lo]]),
                            )
                for c in range(nchunk):
                    out_t = outp.tile([PARTS, CHUNK], mybir.dt.float32, tag="out")
                    for bb in range(bpc):
                        blk_lo = c * CHUNK + bb * BLK
                        pt = ps.tile([PARTS, BLK], mybir.dt.float32, tag="ps")
                        mms = []
                        for s in (0, 1):
                            mms += runs_by_block[s][c * bpc + bb]
                        for k, (o, ln, src) in enumerate(mms):
                            nc.tensor.matmul(
                                pt[:, o - blk_lo : o - blk_lo + ln],
                                wt[:],
                                in_t[:, src : src + ln],
                                start=(k == 0),
                                stop=(k == len(mms) - 1),
                                skip_group_check=True,
                            )
                        nc.vector.affine_then_add(
                            out=out_t[:, bb * BLK : (bb + 1) * BLK],
                            in0=in_t[:, HALO + blk_lo : HALO + blk_lo + BLK],
                            in1=pt[:],
                            scale=0.5,
                            bias=0.0,
                        )
                    # patches for this chunk
                    for o, s, diff, gidx in tile_patches[ti]:
                        if not (c * CHUNK <= o < (c + 1) * CHUNK):
                            continue
                        col = o + HALO - int(tbl[s * HALF + o])
                        t1 = tp.tile([PARTS, 1], mybir.dt.float32, tag="t1")
                        t2 = tp.tile([PARTS, 1], mybir.dt.float32, tag="t2")
                        nc.vector.tensor_tensor(
                            out=t1[:],
                            in0=in_t[:, col - diff : col - diff + 1],
                            in1=in_t[:, col : col + 1],
                            op=mybir.AluOpType.subtract,
                        )
                        nc.vector.tensor_tensor(
                            out=t2[:], in0=t1[:], in1=mkt[:, gidx : gidx + 1],
                            op=mybir.AluOpType.mult,
                        )
                        oc = o - c * CHUNK
                        nc.vector.tensor_tensor(
                            out=out_t[:, oc : oc + 1],
                            in0=out_t[:, oc : oc + 1],
                            in1=t2[:],
                            op=mybir.AluOpType.add,
                        )
                    # store
                    for r in (0, 1):
                        skip = hs0 - h0
                        n_st = nh_t - skip
                        nc.sync.dma_start(
                            AP(y, r * T + hs0 * HALF + c * CHUNK, [[HALF, n_st], [1, CHUNK]]),
                            out_t[r * 64 + skip : r * 64 + nh_t, :],
                        )
    nc.compile()
    return nc, msk_np


_CACHE = {}


def _get_built(nper):
    if nper not in _CACHE:
        _CACHE[nper] = build(nper)
    return _CACHE[nper]


def kernel(x):
    x = np.asarray(x, dtype=np.float32)
    assert x.shape == (B, T_FULL)
    nper = T_FULL // P
    nc, msk_np = _get_built(nper)
    wv = (0.25 * np.eye(PARTS)).astype(np.float32)
    in_maps = [
        {"x": np.ascontiguousarray(x[2 * i : 2 * i + 2]), "w": wv, "msk": msk_np}
        for i in range(N_CORES)
    ]
    res = run_bass_kernel_spmd(nc, in_maps, core_ids=list(range(N_CORES)))
    out = np.concatenate([r["y"] for r in res.results], axis=0)
    return out.astype(np.float32)


if __name__ == "__main__":
    # smoke test on a small number of periods through CoreSim
    from concourse.bass_interp import CoreSim

    nper = 2
    T = nper * P
    nc, msk_np = build(nper)
    rng = np.random.default_rng(0)
    xv = rng.standard_normal((2, T)).astype(np.float32)
    sim = CoreSim(nc, trace=False)
    sim.tensor("x")[:] = xv
    sim.tensor("w")[:] = (0.25 * np.eye(PARTS)).astype(np.float32)
    sim.tensor("msk")[:] = msk_np
    sim.simulate()
    got = sim.tensor("y").copy()
    # reference
    delay = _delay_table(T)
    idx = np.arange(T)[None, :] - delay
    valid = (idx >= 0).astype(np.float32)
    idx = np.maximum(idx, 0)
    wet = (xv[:, idx] * valid[None]).mean(axis=1)
    exp = xv * 0.5 + wet * 0.5
    err = np.abs(got - exp).max()
    print("smoke absmax err:", err, "rel:", err / np.abs(exp).max())
